# Initial kernel scaffold
#
"""Trainium2 Bass kernel for EnhancedSNN (2-layer LIF spiking net, 50 steps).

Math:
    cur1 = x @ w1.T + b1                      (loop-invariant)
    m_0 = cur1;  m_t = b*m_{t-1} + cur1 - spk_{t-1};  spk_t = (m_t > 1)
    cur2_t = spk1_t @ w2.T + b2               (layer 2 analogous, cur2 varies)

Transform (layer 1): K = cur1/(1-b), z = m - K  =>  z_t = b*z_{t-1} - spk_{t-1},
    spk_t = (z_t > thr),  thr = 1 - K = 1 + z_0/b,  z_0 = -(b/(1-b)) * cur1.
This removes the per-step "+cur1" elementwise pass entirely.

Sharding: data-parallel over batch, 8 cores x 256 batch rows. Weights replicated.
Device layout: [neuron on partitions, batch on free]; spikes emitted as bf16
(0/1 exact) and transposed/cast to f32 on host.
"""

import sys

sys.path.insert(0, "/opt/trn_rl_repo")

import numpy as np
import ml_dtypes

import concourse.bass as bass
import concourse.mybir as mybir
from concourse.tile import TileContext
from concourse import bass_utils

F32 = mybir.dt.float32
BF16 = mybir.dt.bfloat16
NP_BF16 = ml_dtypes.bfloat16

T = 50
BETA = 0.9
B, NI, NH, NO = 2048, 1024, 1024, 10
NCORES = 8
BC = B // NCORES  # 256 batch rows per core
NHC = NH // 128  # 8 neuron chunks

# Set by test harness to collect a profile; kernel() stores results here.
TRACE = False
LAST_RESULTS = None

_CACHED = None  # (nc,) compiled program cache


def _split_hilo(a32):
    hi = a32.astype(NP_BF16)
    lo = (a32 - hi.astype(np.float32)).astype(NP_BF16)
    return hi, lo


def _build_nc():
    gt = mybir.AluOpType.is_gt
    mult = mybir.AluOpType.mult
    sub = mybir.AluOpType.subtract
    add = mybir.AluOpType.add
    Copy = mybir.ActivationFunctionType.Copy

    beta = float(np.float32(BETA))
    inv_beta = float(np.float64(1.0) / np.float64(np.float32(BETA)))

    nc = bass.Bass()
    xT_d = nc.dram_tensor("xT", [NI, BC], F32, kind="ExternalInput")
    w1s_d = nc.dram_tensor("w1s", [NI, NH], F32, kind="ExternalInput")
    b1s_d = nc.dram_tensor("b1s", [1, NH], F32, kind="ExternalInput")
    w2h_d = nc.dram_tensor("w2h", [NH, NO], BF16, kind="ExternalInput")
    w2l_d = nc.dram_tensor("w2l", [NH, NO], BF16, kind="ExternalInput")
    b2h_d = nc.dram_tensor("b2h", [1, NO], BF16, kind="ExternalInput")
    b2l_d = nc.dram_tensor("b2l", [1, NO], BF16, kind="ExternalInput")
    spk1_d = nc.dram_tensor("spk1", [T, NH, BC], BF16, kind="ExternalOutput")
    spk2_d = nc.dram_tensor("spk2", [T, BC, NO], BF16, kind="ExternalOutput")

    with TileContext(nc) as tc:
        with (
            tc.tile_pool(name="wpool", bufs=1) as wpool,
            tc.tile_pool(name="spool", bufs=1) as spool,
            tc.tile_pool(name="kpool", bufs=3) as kpool,
            tc.tile_pool(name="pspool", bufs=2, space="PSUM") as pspool,
            tc.tile_pool(name="ps2pool", bufs=2, space="PSUM") as ps2pool,
        ):
            # ---- load weights / inputs ----
            w1s_t = wpool.tile([128, NHC * NH], F32)
            for c in range(NHC):
                nc.sync.dma_start(
                    out=w1s_t[:, c * NH : (c + 1) * NH],
                    in_=w1s_d[c * 128 : (c + 1) * 128, :],
                )
            xT_t = wpool.tile([128, NHC * BC], F32)
            for c in range(NHC):
                nc.sync.dma_start(
                    out=xT_t[:, c * BC : (c + 1) * BC],
                    in_=xT_d[c * 128 : (c + 1) * 128, :],
                )
            b1s_t = wpool.tile([1, NH], F32)
            nc.sync.dma_start(out=b1s_t[:], in_=b1s_d[:, :])
            w2h_t = wpool.tile([128, NHC * NO], BF16)
            w2l_t = wpool.tile([128, NHC * NO], BF16)
            for c in range(NHC):
                nc.sync.dma_start(
                    out=w2h_t[:, c * NO : (c + 1) * NO],
                    in_=w2h_d[c * 128 : (c + 1) * 128, :],
                )
                nc.sync.dma_start(
                    out=w2l_t[:, c * NO : (c + 1) * NO],
                    in_=w2l_d[c * 128 : (c + 1) * 128, :],
                )
            b2h_t = wpool.tile([1, NO], BF16)
            nc.sync.dma_start(out=b2h_t[:], in_=b2h_d[:, :])
            b2l_t = wpool.tile([1, NO], BF16)
            nc.sync.dma_start(out=b2l_t[:], in_=b2l_d[:, :])
            ones_f = wpool.tile([1, BC], F32)
            nc.vector.memset(ones_f[:], 1.0)
            ones_b = wpool.tile([1, 128], BF16)
            nc.vector.memset(ones_b[:], 1.0)

            # ---- persistent state ----
            z_t = spool.tile([128, NHC * BC], F32)
            thr_t = spool.tile([128, NHC * BC], F32)
            m2_t = spool.tile([128, 2 * NO], F32)
            nc.vector.memset(m2_t[:], 0.0)
            spk2_zero = spool.tile([128, 2 * NO], BF16)
            nc.vector.memset(spk2_zero[:], 0.0)
            spk2_all = spool.tile([128, T * 2 * NO], BF16)

            # ---- fc1: z_0 = -(b/(1-b)) * (x @ w1.T + b1), thr = 1 + z_0/b ----
            for h in range(NHC):
                ps = pspool.tile([128, BC], F32, tag="fc1")
                for c in range(NHC):
                    nc.tensor.matmul(
                        ps[:],
                        lhsT=w1s_t[:, c * NH + h * 128 : c * NH + (h + 1) * 128],
                        rhs=xT_t[:, c * BC : (c + 1) * BC],
                        start=(c == 0),
                        stop=False,
                    )
                nc.tensor.matmul(
                    ps[:],
                    lhsT=b1s_t[:, h * 128 : (h + 1) * 128],
                    rhs=ones_f[:],
                    start=False,
                    stop=True,
                )
                nc.scalar.activation(z_t[:, h * BC : (h + 1) * BC], ps[:], Copy)
                nc.scalar.activation(
                    thr_t[:, h * BC : (h + 1) * BC],
                    ps[:],
                    Copy,
                    bias=1.0,
                    scale=inv_beta,
                )

            # ---- temporal loop ----
            spk2_prev = spk2_zero[:]
            for t in range(T):
                spk_t = kpool.tile([128, NHC * BC], BF16, tag="spk")
                nc.vector.tensor_tensor(out=spk_t[:], in0=z_t[:], in1=thr_t[:], op=gt)
                nc.sync.dma_start(
                    out=spk1_d[t].rearrange("(c p) b -> p c b", p=128),
                    in_=spk_t[:].rearrange("p (c b) -> p c b", b=BC),
                )
                if t < T - 1:
                    nc.vector.scalar_tensor_tensor(
                        out=z_t[:], in0=z_t[:], scalar=beta, in1=spk_t[:],
                        op0=mult, op1=sub,
                    )
                # fc2: cur2[b, o] accumulated per batch-half (spk slices as weights)
                ps2 = []
                for half in range(2):
                    p2 = ps2pool.tile([128, NO], F32, tag=f"cur2_{half}")
                    ps2.append(p2)
                    nc.tensor.matmul(
                        p2[:], lhsT=ones_b[:], rhs=b2h_t[:], start=True, stop=False
                    )
                    nc.tensor.matmul(
                        p2[:], lhsT=ones_b[:], rhs=b2l_t[:], start=False, stop=False
                    )
                    for c in range(NHC):
                        lhs = spk_t[:, c * BC + half * 128 : c * BC + half * 128 + 128]
                        nc.tensor.matmul(
                            p2[:], lhsT=lhs, rhs=w2h_t[:, c * NO : (c + 1) * NO],
                            start=False, stop=False,
                        )
                        nc.tensor.matmul(
                            p2[:], lhsT=lhs, rhs=w2l_t[:, c * NO : (c + 1) * NO],
                            start=False, stop=(c == NHC - 1),
                        )
                # layer-2 LIF update: m2 = b*m2 - spk2_prev + cur2 ; spk2 = m2 > 1
                nc.vector.scalar_tensor_tensor(
                    out=m2_t[:], in0=m2_t[:], scalar=beta, in1=spk2_prev,
                    op0=mult, op1=sub,
                )
                for half in range(2):
                    sl = m2_t[:, half * NO : (half + 1) * NO]
                    nc.vector.tensor_tensor(out=sl, in0=sl, in1=ps2[half][:], op=add)
                spk2_slice = spk2_all[:, t * 2 * NO : (t + 1) * 2 * NO]
                nc.vector.tensor_scalar(
                    out=spk2_slice, in0=m2_t[:], scalar1=1.0, scalar2=None, op0=gt
                )
                spk2_prev = spk2_slice

            # ---- store spk2: sbuf [p, (t h o)] -> dram [t, (h p), o] ----
            src4 = spk2_all[:].rearrange("p (t h o) -> p t h o", t=T, h=2, o=NO)
            dst4 = spk2_d[:, :, :].rearrange("t (h p) o -> h p t o", h=2)
            for half in range(2):
                nc.sync.dma_start(out=dst4[half], in_=src4[:, :, half, :])

    return nc


def _get_nc():
    global _CACHED
    if _CACHED is None:
        _CACHED = _build_nc()
    return _CACHED


def kernel(x, w1, b1, w2, b2):
    global LAST_RESULTS
    x = np.ascontiguousarray(np.asarray(x, np.float32))
    w1 = np.asarray(w1, np.float32)
    b1 = np.asarray(b1, np.float32)
    w2 = np.asarray(w2, np.float32)
    b2 = np.asarray(b2, np.float32)

    beta32 = np.float64(np.float32(BETA))
    factor = beta32 / (np.float64(1.0) - beta32)  # b/(1-b) with f32 beta

    w1s = np.ascontiguousarray((-factor * w1.T.astype(np.float64)).astype(np.float32))
    b1s = (-factor * b1.astype(np.float64)).astype(np.float32)[None, :]
    w2h, w2l = _split_hilo(np.ascontiguousarray(w2.T, dtype=np.float32))
    b2h, b2l = _split_hilo(b2)
    b2h, b2l = b2h[None, :], b2l[None, :]

    in_maps = []
    for c in range(NCORES):
        xT_c = np.ascontiguousarray(x[c * BC : (c + 1) * BC].T)
        in_maps.append(
            {
                "xT": xT_c,
                "w1s": w1s,
                "b1s": b1s,
                "w2h": w2h,
                "w2l": w2l,
                "b2h": b2h,
                "b2l": b2l,
            }
        )

    nc = _get_nc()
    res = bass_utils.run_bass_kernel_spmd(
        nc, in_maps, core_ids=list(range(NCORES)), trace=TRACE
    )
    LAST_RESULTS = res

    spikes1 = np.empty((T, B, NH), np.float32)
    spikes2 = np.empty((T, B, NO), np.float32)
    for c in range(NCORES):
        s1 = np.asarray(res.results[c]["spk1"])  # [T, NH, BC] bf16
        spikes1[:, c * BC : (c + 1) * BC, :] = s1.astype(np.float32).transpose(0, 2, 1)
        s2 = np.asarray(res.results[c]["spk2"])  # [T, BC, NO] bf16
        spikes2[:, c * BC : (c + 1) * BC, :] = s2.astype(np.float32)
    return spikes1, spikes2


# revision 14
# speedup vs baseline: 1.3678x; 1.3678x over previous
"""Trainium2 Bass kernel for EnhancedSNN (2-layer LIF spiking net, 50 steps).

Math:
    cur1 = x @ w1.T + b1                      (loop-invariant)
    m_0 = cur1;  m_t = b*m_{t-1} + cur1 - spk_{t-1};  spk_t = (m_t > 1)
    cur2_t = spk1_t @ w2.T + b2               (layer 2 analogous, cur2 varies)

Transform (layer 1): K = cur1/(1-b), z = m - K  =>  z_t = b*z_{t-1} - spk_{t-1},
    spk_t = (z_t > thr),  thr = 1 - K = 1 + z_0/b,  z_0 = -(b/(1-b)) * cur1.
This removes the per-step "+cur1" elementwise pass entirely.

Engine split per step (z lives in PSUM, 4 banks of [128,512]):
    VectorE:  spk = (z > thr)            (tensor_tensor is_gt, PSUM src)
    ScalarE:  z *= b                     (activation Copy, in-place PSUM)
    TensorE:  z += (-I) @ spk            (accumulating matmul; has_written
              bits stay set from fc1 since only TensorE touches them)
    TensorE:  cur2 = spk @ w2.T + b2     (spk slices as bf16 weights, hi/lo)

Sharding: data-parallel over batch, 8 cores x 256 batch rows. Weights replicated.
Device layout: [neuron on partitions, batch on free]; spikes emitted as bf16
(0/1 exact) and transposed/cast to f32 on host.
"""

import sys

sys.path.insert(0, "/opt/trn_rl_repo")

import numpy as np
import ml_dtypes

import concourse.bass as bass
import concourse.bacc as bacc
import concourse.mybir as mybir
from concourse.tile import TileContext
from concourse import bass_utils

F32 = mybir.dt.float32
BF16 = mybir.dt.bfloat16
NP_BF16 = ml_dtypes.bfloat16

T = 50
BETA = 0.9
B, NI, NH, NO = 2048, 1024, 1024, 10
NCORES = 8
BC = B // NCORES  # 256 batch rows per core
NHC = NH // 128  # 8 neuron chunks
NBANK = 4  # z PSUM banks, each [128, 512] f32 covering two neuron chunks

# Set by test harness; kernel() stores BassKernelResults here.
TRACE = False
LAST_RESULTS = None

_CACHED = {}  # repeats -> built program


def _split_hilo(a32):
    hi = a32.astype(NP_BF16)
    lo = (a32 - hi.astype(np.float32)).astype(NP_BF16)
    return hi, lo


def _build_nc(repeats=1):
    gt = mybir.AluOpType.is_gt
    mult = mybir.AluOpType.mult
    sub = mybir.AluOpType.subtract
    add = mybir.AluOpType.add
    Copy = mybir.ActivationFunctionType.Copy

    beta = float(np.float32(BETA))
    inv_beta = float(np.float64(1.0) / np.float64(np.float32(BETA)))

    nc = bacc.Bacc("TRN2", target_bir_lowering=False)
    xT_d = nc.dram_tensor("xT", [NI, BC], F32, kind="ExternalInput")
    w1s_d = nc.dram_tensor("w1s", [NI, NH], F32, kind="ExternalInput")
    b1s_d = nc.dram_tensor("b1s", [1, NH], F32, kind="ExternalInput")
    w2h_d = nc.dram_tensor("w2h", [NH, NO], BF16, kind="ExternalInput")
    w2l_d = nc.dram_tensor("w2l", [NH, NO], BF16, kind="ExternalInput")
    b2h_d = nc.dram_tensor("b2h", [1, NO], BF16, kind="ExternalInput")
    b2l_d = nc.dram_tensor("b2l", [1, NO], BF16, kind="ExternalInput")
    negI_d = nc.dram_tensor("negI", [128, 128], BF16, kind="ExternalInput")
    spk1_d = nc.dram_tensor("spk1", [T, NH, BC], BF16, kind="ExternalOutput")
    spk2_d = nc.dram_tensor("spk2", [T, BC, NO], BF16, kind="ExternalOutput")

    with TileContext(nc) as tc:
        with (
            tc.tile_pool(name="wpool", bufs=1) as wpool,
            tc.tile_pool(name="spool", bufs=1) as spool,
            tc.tile_pool(name="kpool", bufs=3) as kpool,
            tc.tile_pool(name="zpool", bufs=1, space="PSUM") as zpool,
            tc.tile_pool(name="ps2pool", bufs=2, space="PSUM") as ps2pool,
        ):
            for rep in range(repeats):
                # ---- load weights / inputs ----
                w1s_t = wpool.tile([128, NHC * NH], F32)
                nc.gpsimd.dma_start(
                    out=w1s_t[:].rearrange("p (c n) -> p c n", c=NHC),
                    in_=w1s_d[:, :].rearrange("(c p) n -> p c n", p=128),
                )
                xT_t = wpool.tile([128, NHC * BC], F32)
                nc.gpsimd.dma_start(
                    out=xT_t[:].rearrange("p (c b) -> p c b", c=NHC),
                    in_=xT_d[:, :].rearrange("(c p) b -> p c b", p=128),
                )
                b1s_t = wpool.tile([1, NH], F32)
                nc.gpsimd.dma_start(out=b1s_t[:], in_=b1s_d[:, :])
                w2h_t = wpool.tile([128, NHC * NO], BF16)
                nc.gpsimd.dma_start(
                    out=w2h_t[:].rearrange("p (c o) -> p c o", c=NHC),
                    in_=w2h_d[:, :].rearrange("(c p) o -> p c o", p=128),
                )
                w2l_t = wpool.tile([128, NHC * NO], BF16)
                nc.gpsimd.dma_start(
                    out=w2l_t[:].rearrange("p (c o) -> p c o", c=NHC),
                    in_=w2l_d[:, :].rearrange("(c p) o -> p c o", p=128),
                )
                b2h_t = wpool.tile([1, NO], BF16)
                nc.gpsimd.dma_start(out=b2h_t[:], in_=b2h_d[:, :])
                b2l_t = wpool.tile([1, NO], BF16)
                nc.gpsimd.dma_start(out=b2l_t[:], in_=b2l_d[:, :])
                negI_t = wpool.tile([128, 128], BF16)
                nc.gpsimd.dma_start(out=negI_t[:], in_=negI_d[:, :])
                ones_f = wpool.tile([1, BC], F32)
                nc.vector.memset(ones_f[:], 1.0)
                ones_b = wpool.tile([1, 128], BF16)
                nc.vector.memset(ones_b[:], 1.0)

                # ---- persistent state ----
                z_ps = zpool.tile([128, NHC * BC], F32)  # 4 PSUM banks
                thr_t = spool.tile([128, NHC * BC], F32)
                m2_t = spool.tile([128, 2 * NO], F32)
                nc.vector.memset(m2_t[:], 0.0)
                spk2_zero = spool.tile([128, 2 * NO], BF16)
                nc.vector.memset(spk2_zero[:], 0.0)
                spk2_all = spool.tile([128, T * 2 * NO], BF16)

                # ---- fc1 into z PSUM: z_0 = -(b/(1-b)) * (x @ w1.T + b1) ----
                # start=True only on the first matmul of each BANK (even h):
                # it clears has_written for the whole bank; the odd-h region's
                # first matmul then overwrites-and-sets via clear bits. After
                # fc1 every bank has all bits set, so per-step accumulating
                # matmuls add onto ACT-scaled values.
                for h in range(NHC):
                    zslice = z_ps[:, h * BC : (h + 1) * BC]
                    for c in range(NHC):
                        nc.tensor.matmul(
                            zslice,
                            lhsT=w1s_t[:, c * NH + h * 128 : c * NH + (h + 1) * 128],
                            rhs=xT_t[:, c * BC : (c + 1) * BC],
                            start=(h % 2 == 0 and c == 0),
                            stop=False,
                            skip_group_check=True,
                        )
                    nc.tensor.matmul(
                        zslice,
                        lhsT=b1s_t[:, h * 128 : (h + 1) * 128],
                        rhs=ones_f[:],
                        start=False,
                        stop=(h % 2 == 1),
                        skip_group_check=True,
                    )
                    nc.scalar.activation(
                        thr_t[:, h * BC : (h + 1) * BC],
                        zslice,
                        Copy,
                        bias=1.0,
                        scale=inv_beta,
                    )

                # ---- temporal loop ----
                spk2_prev = spk2_zero[:]
                for t in range(T):
                    spk_t = kpool.tile([128, NHC * BC], BF16, tag="spk")
                    for k in range(NBANK):
                        sl = slice(k * 512, (k + 1) * 512)
                        nc.vector.tensor_tensor(
                            out=spk_t[:, sl], in0=z_ps[:, sl], in1=thr_t[:, sl], op=gt
                        )
                    nc.sync.dma_start(
                        out=spk1_d[t].rearrange("(c p) b -> p c b", p=128),
                        in_=spk_t[:].rearrange("p (c b) -> p c b", b=BC),
                    )
                    if t < T - 1:
                        for k in range(NBANK):
                            sl = slice(k * 512, (k + 1) * 512)
                            nc.scalar.activation(
                                z_ps[:, sl], z_ps[:, sl], Copy, scale=beta
                            )
                            nc.tensor.matmul(
                                z_ps[:, sl],
                                lhsT=negI_t[:],
                                rhs=spk_t[:, sl],
                                start=False,
                                stop=True,
                                skip_group_check=True,
                            )
                    # fc2: cur2[b, o] per batch-half; spk slices as bf16 weights
                    ps2 = []
                    for half in range(2):
                        p2 = ps2pool.tile([128, NO], F32, tag=f"cur2_{half}")
                        ps2.append(p2)
                        for c in range(NHC):
                            lhs = spk_t[
                                :, c * BC + half * 128 : c * BC + half * 128 + 128
                            ]
                            nc.tensor.matmul(
                                p2[:], lhsT=lhs, rhs=w2h_t[:, c * NO : (c + 1) * NO],
                                start=(c == 0), stop=False,
                            )
                            nc.tensor.matmul(
                                p2[:], lhsT=lhs, rhs=w2l_t[:, c * NO : (c + 1) * NO],
                                start=False, stop=False,
                            )
                        nc.tensor.matmul(
                            p2[:], lhsT=ones_b[:], rhs=b2h_t[:], start=False, stop=False
                        )
                        nc.tensor.matmul(
                            p2[:], lhsT=ones_b[:], rhs=b2l_t[:], start=False, stop=True
                        )
                    # layer-2 LIF update mirroring the reference's float-op
                    # order: m2 = (b*m2 + cur2) - spk2_prev ; spk2 = m2 > 1
                    for half in range(2):
                        sl2 = m2_t[:, half * NO : (half + 1) * NO]
                        nc.vector.scalar_tensor_tensor(
                            out=sl2, in0=sl2, scalar=beta, in1=ps2[half][:],
                            op0=mult, op1=add,
                        )
                    nc.vector.tensor_tensor(
                        out=m2_t[:], in0=m2_t[:], in1=spk2_prev, op=sub
                    )
                    spk2_slice = spk2_all[:, t * 2 * NO : (t + 1) * 2 * NO]
                    nc.vector.tensor_scalar(
                        out=spk2_slice, in0=m2_t[:], scalar1=1.0, scalar2=None, op0=gt
                    )
                    spk2_prev = spk2_slice

                # ---- store spk2: sbuf [p, (t h o)] -> dram [t, (h p), o] ----
                src4 = spk2_all[:].rearrange("p (t h o) -> p t h o", t=T, h=2, o=NO)
                dst4 = spk2_d[:, :, :].rearrange("t (h p) o -> h p t o", h=2)
                for half in range(2):
                    nc.sync.dma_start(out=dst4[half], in_=src4[:, :, half, :])

    nc.finalize()
    return nc


def _get_nc(repeats=1):
    if repeats not in _CACHED:
        _CACHED[repeats] = _build_nc(repeats)
    return _CACHED[repeats]


def _prepare_inmaps(x, w1, b1, w2, b2):
    x = np.ascontiguousarray(np.asarray(x, np.float32))
    w1 = np.asarray(w1, np.float32)
    b1 = np.asarray(b1, np.float32)
    w2 = np.asarray(w2, np.float32)
    b2 = np.asarray(b2, np.float32)

    beta32 = np.float64(np.float32(BETA))
    factor = beta32 / (np.float64(1.0) - beta32)  # b/(1-b) with f32 beta

    w1s = np.ascontiguousarray((-factor * w1.T.astype(np.float64)).astype(np.float32))
    b1s = (-factor * b1.astype(np.float64)).astype(np.float32)[None, :]
    w2h, w2l = _split_hilo(np.ascontiguousarray(w2.T, dtype=np.float32))
    b2h, b2l = _split_hilo(b2)
    b2h, b2l = b2h[None, :], b2l[None, :]
    negI = (-np.eye(128, dtype=np.float32)).astype(NP_BF16)

    in_maps = []
    for c in range(NCORES):
        xT_c = np.ascontiguousarray(x[c * BC : (c + 1) * BC].T)
        in_maps.append(
            {
                "xT": xT_c,
                "w1s": w1s,
                "b1s": b1s,
                "w2h": w2h,
                "w2l": w2l,
                "b2h": b2h,
                "b2l": b2l,
                "negI": negI,
            }
        )
    return in_maps


def kernel(x, w1, b1, w2, b2):
    global LAST_RESULTS
    in_maps = _prepare_inmaps(x, w1, b1, w2, b2)
    nc = _get_nc()
    res = bass_utils.run_bass_kernel_spmd(
        nc, in_maps, core_ids=list(range(NCORES)), trace=TRACE
    )
    LAST_RESULTS = res

    spikes1 = np.empty((T, B, NH), np.float32)
    spikes2 = np.empty((T, B, NO), np.float32)
    for c in range(NCORES):
        s1 = np.asarray(res.results[c]["spk1"])  # [T, NH, BC] bf16
        spikes1[:, c * BC : (c + 1) * BC, :] = s1.astype(np.float32).transpose(0, 2, 1)
        s2 = np.asarray(res.results[c]["spk2"])  # [T, BC, NO] bf16
        spikes2[:, c * BC : (c + 1) * BC, :] = s2.astype(np.float32)
    return spikes1, spikes2
